# revision 2
# baseline (speedup 1.0000x reference)
"""CTMHead kernel: 8-way sharded execution on Trainium2 NeuronCores.

Sharding: 8 cores = 4 batches x 2 sequence-halves. All per-(b,s) ops are
embarrassingly parallel; causal attention needs the full-prefix K/V, so each
core recomputes K/V for its batch's full sequence (cheap preamble) and runs
the 10-tick recurrence only for its own 512 query positions.
"""
import math

import jax
import jax.numpy as jnp
import numpy as np

T_TICKS = 10
N_HEADS = 8
B, S, L0, HID, D, M, DH, P = 4, 1024, 2048, 512, 256, 25, 32, 512
N_CORES = 8
HALF = S // 2  # 512 tokens per core


def _ln(x, g, b, axes=(-1,)):
    mu = jnp.mean(x, axis=axes, keepdims=True)
    var = jnp.mean((x - mu) ** 2, axis=axes, keepdims=True)
    return (x - mu) * jax.lax.rsqrt(var + 1e-5) * g + b


def _shard_fn(l0_seq_b, q_lo_onehot, params):
    """Compute outputs for one (batch, seq-half) shard.

    l0_seq_b: [S, L0] full sequence of this core's batch.
    q_lo_onehot: [2] one-hot over {first-half, second-half}; used to build the
      causal mask/offset without dynamic shapes (same program on all cores).
    """
    p = params
    hd = HID // N_HEADS
    # ---- preamble: shared K/V over the full sequence ----
    x = _ln(l0_seq_b, p['in_ln_g'], p['in_ln_b'])
    kv = x @ p['kvW'] + p['kvb']
    k = (kv @ p['kW'] + p['kb']).reshape(S, N_HEADS, hd).transpose(1, 0, 2)
    v = (kv @ p['vW'] + p['vb']).reshape(S, N_HEADS, hd).transpose(1, 0, 2)

    # this core's query offset: 0 or HALF
    q_off = q_lo_onehot[1] * HALF  # scalar int32
    q_pos = q_off + jnp.arange(HALF)          # [HALF] global positions
    causal = q_pos[:, None] >= jnp.arange(S)[None, :]  # [HALF, S]

    zst = jnp.broadcast_to(p['z_init'], (HALF, D))
    a_hist = jnp.broadcast_to(p['a_hist_init'], (HALF, D, M))
    alpha_a = jnp.zeros((HALF, P), jnp.float32)
    alpha_o = jnp.zeros((HALF, P), jnp.float32)
    ga = jax.nn.sigmoid(p['ra_raw'])
    go = jax.nn.sigmoid(p['ro_raw'])
    inv_sqrt_m = 1.0 / math.sqrt(M)
    inv_sqrt_hd = 1.0 / math.sqrt(hd)

    def tick(carry, _):
        zst, a_hist, alpha_a, alpha_o = carry
        prod = zst[:, p['ia_l']] * zst[:, p['ia_r']]
        alpha_a = ga * alpha_a + (1.0 - ga) * prod
        sync_a = alpha_a  # beta_a stays exactly 1.0
        q = (sync_a @ p['qW'] + p['qb']).reshape(HALF, N_HEADS, hd).transpose(1, 0, 2)
        scores = jnp.einsum('hqd,hkd->hqk', q, k) * inv_sqrt_hd
        scores = jnp.where(causal[None, :, :], scores, -1e9)
        attn = jax.nn.softmax(scores, axis=-1)
        ao = jnp.einsum('hqk,hkd->hqd', attn, v).transpose(1, 0, 2).reshape(HALF, HID)
        o = ao @ p['oaW'] + p['oab']
        syn_in = jnp.concatenate([zst, o], axis=-1)
        h = jnp.tanh(_ln(syn_in @ p['syn_w1'] + p['syn_b1'],
                         p['syn_ln_g'], p['syn_ln_b']))
        a_t = jnp.tanh(h @ p['syn_w2'] + p['syn_b2'])
        a_hist = jnp.concatenate([a_hist[..., 1:], a_t[..., None]], axis=-1)
        a_sc = a_hist * inv_sqrt_m
        nh = jnp.einsum('sdm,mhd->shd', a_sc, p['nlm_w1']) + p['nlm_b1']
        nh = jnp.tanh(_ln(nh, p['nlm_ln_g'], p['nlm_ln_b'], axes=(-2, -1)))
        zst = jnp.tanh(jnp.einsum('shd,hd->sd', nh, p['nlm_w2']) + p['nlm_b2'])
        prod_o = zst[:, p['io_l']] * zst[:, p['io_r']]
        alpha_o = go * alpha_o + (1.0 - go) * prod_o
        sync_o = alpha_o
        return (zst, a_hist, alpha_a, alpha_o), sync_o

    (zst, _, _, alpha_o), _ = jax.lax.scan(
        tick, (zst, a_hist, alpha_a, alpha_o), None, length=T_TICKS)
    sync_o = alpha_o

    h_out = jax.nn.gelu(_ln(sync_o @ p['op1W'] + p['op1b'],
                            p['op_ln_g'], p['op_ln_b']), approximate=False)
    features = _ln(h_out @ p['op2W'] + p['op2b'], p['fin_ln_g'], p['fin_ln_b'])
    certainty = jax.nn.sigmoid(h_out @ p['certW'] + p['certb'])
    value = h_out @ p['valW'] + p['valb']
    synchronization = jnp.abs(zst).mean(axis=-1)
    return features, synchronization, certainty, value


_COMPILED = {}


def _get_compiled():
    if 'fn' not in _COMPILED:
        devs = jax.devices()[:N_CORES]
        mesh = jax.sharding.Mesh(np.array(devs), ('x',))
        pspec = jax.sharding.PartitionSpec('x')

        def spmd(l0_stack, onehot_stack, params):
            fs, ss, cs, vs = jax.vmap(
                lambda l, oh: _shard_fn(l, oh, params))(l0_stack, onehot_stack)
            return fs, ss, cs, vs

        fn = jax.jit(
            spmd,
            in_shardings=(
                jax.sharding.NamedSharding(mesh, pspec),
                jax.sharding.NamedSharding(mesh, pspec),
                jax.sharding.NamedSharding(mesh, jax.sharding.PartitionSpec()),
            ),
            out_shardings=jax.sharding.NamedSharding(mesh, pspec),
        )
        _COMPILED['fn'] = fn
    return _COMPILED['fn']


def kernel(**inputs):
    inp = {k: np.asarray(v) for k, v in inputs.items()}
    l0_seq = inp.pop('l0_seq')
    params = {k: jnp.asarray(v) for k, v in inp.items()}

    # stack of 8 shards: core i handles (b = i // 2, half = i % 2)
    l0_stack = np.stack([l0_seq[i // 2] for i in range(N_CORES)])  # [8, S, L0]
    onehot = np.stack([np.eye(2, dtype=np.float32)[i % 2] for i in range(N_CORES)])

    fn = _get_compiled()
    fs, ss, cs, vs = fn(jnp.asarray(l0_stack), jnp.asarray(onehot), params)
    fs = np.asarray(fs)  # [8, HALF, L0]
    ss = np.asarray(ss)  # [8, HALF]
    cs = np.asarray(cs)  # [8, HALF, 1]
    vs = np.asarray(vs)  # [8, HALF, 1]

    features = np.zeros((B, S, L0), np.float32)
    synchronization = np.zeros((B, S), np.float32)
    certainty = np.zeros((B, S, 1), np.float32)
    value = np.zeros((B, S, 1), np.float32)
    for i in range(N_CORES):
        b, half = i // 2, i % 2
        sl = slice(half * HALF, (half + 1) * HALF)
        features[b, sl] = fs[i]
        synchronization[b, sl] = ss[i]
        certainty[b, sl] = cs[i]
        value[b, sl] = vs[i]
    return features, synchronization, certainty, value
